# revision 7
# baseline (speedup 1.0000x reference)
"""Trainium2 Bass kernel for nn_LocalExperts (MoE expert-parallel FFN).

Reference computation (per full input):
    x  [T=16384, D=1024] -> reshape [E=8, C=2048, D]
    h  = gelu(x @ w1[e] + b1[e])     w1 [E, D, F=4096]
    y  = h @ w2[e] + b2[e]           w2 [E, F, D]
    out[T, D]

Sharding: expert parallelism across 8 NeuronCores. Expert e's tokens are
exactly rows [e*C:(e+1)*C] of the input, so core e gets that token slice
plus w1[e], b1[e], w2[e], b2[e]. No collectives needed; outputs are
concatenated on the host.

Host-side layout prep (free w.r.t. HW exec time): the token slice is
passed pre-transposed as xt [D, C] so the contraction dim D lands on
SBUF partitions via plain DMA — no PE transposes on device. b1 is
passed as b1t [128, F/128] (per-partition bias of each f-tile).

Per-core kernel (C=2048 tokens, one expert), PE runs matmuls only:
  - Two token passes of CP=1024 (xt/yacc double-buffered across passes);
    per pass, loop F in chunks of FC=512:
      GEMM1: Ht[f,c] = gelu(W1c-tiles.T @ Xt + b1)  (PSUM acc over D,
                                                     ACT drain w/ bias)
      GEMM2: Yacc[c,d] += Ht-tiles.T @ W2c          (PSUM acc over FC,
                                                     DVE acc over chunks)
  - Weight chunks stream with 1-chunk lookahead over the flat
    (pass, chunk) sequence; w1 on the scalar HWDGE ring, w2 on sync,
    so startup and steady-state loads run on both rings in parallel.
  - Matmuls run as float32r (full PE rate at N=512, ~TF32 precision,
    fp32 PSUM accumulation).
"""

import os
from contextlib import ExitStack

import numpy as np

import concourse.bass as bass
import concourse.tile as tile
from concourse import bacc
from concourse import mybir
from concourse.bass import ds, ts
from concourse.bass_utils import run_bass_kernel_spmd
from concourse.masks import make_identity

AFT = mybir.ActivationFunctionType

E = 8
D = 1024
F = 4096
T = 16384
C = T // E          # tokens per core
P = 128

N_PASS = 2          # token passes (halves SBUF residency of Xt/Yacc)
CP = C // N_PASS    # tokens per pass
FC = 512            # F chunk per iteration
NFREE = 512         # matmul moving free dim (one PSUM bank of fp32)

D_T = D // P        # 8 d-tiles
F_T = F // P        # 32 f-tiles
FC_T = FC // P      # 4 f-tiles per chunk
N_FC = F // FC      # 8 chunks per pass

# "f32r" (default): fp32 data, float32r matmul (full PE rate, ~TF32 mantissa)
# "f32": plain fp32 matmul (4 cycles/row, ~4x slower PE)
MM_MODE = os.environ.get("KERNEL_MM_MODE", "f32r")
# test-only: CoreSim lacks Gelu; "tanh" swaps the activation for sim gating
ACT_FN = os.environ.get("KERNEL_ACT", "gelu")


def _emit(ctx: ExitStack, tc: tile.TileContext, xt_d, w1, b1t_d, w2, b2, y):
    nc = tc.nc
    f32 = mybir.dt.float32
    f32r = mybir.dt.float32r
    # Matmul operand tiles live natively in this dtype; fp32r DRAM tensors
    # DMA straight into fp32r tiles (bit-identical to fp32).
    mm_dt = f32r if MM_MODE == "f32r" else f32

    consts = ctx.enter_context(tc.tile_pool(name="consts", bufs=1))
    xt_pool = ctx.enter_context(tc.tile_pool(name="xt", bufs=2))
    # yacc is only ever read by DVE (final add goes to ybounce, not back to
    # yacc), so a single buffer has no cross-pass WAR hazard with the y DMA.
    yacc_pool = ctx.enter_context(tc.tile_pool(name="yacc", bufs=1))
    yb_pool = ctx.enter_context(tc.tile_pool(name="yb", bufs=3))
    w1_pool = ctx.enter_context(tc.tile_pool(name="w1c", bufs=2))
    w2_pool = ctx.enter_context(tc.tile_pool(name="w2c", bufs=2))
    ht_pool = ctx.enter_context(tc.tile_pool(name="ht", bufs=1))
    mm_psum = ctx.enter_context(tc.tile_pool(name="mmp", bufs=8, space="PSUM"))

    identity = consts.tile([P, P], f32)
    make_identity(nc, identity[:])
    b1t = consts.tile([P, F_T], f32)
    b2b = consts.tile([P, D], f32)

    # Warm the PE HAM clock (cold 1.2GHz -> 2.4GHz needs ~3.4us of activity)
    # during the initial DMA wait, using identity matmuls.
    warm_ps = mm_psum.tile([P, NFREE], f32, tag="mm")
    for _ in range(24):
        nc.tensor.matmul(warm_ps[:, :P], lhsT=identity[:], rhs=identity[:],
                         start=True, stop=True)

    xt_r = xt_d.rearrange("(dt p) c -> p dt c", p=P)  # [128, 8, 2048]
    w1_r = w1.rearrange("(do p) f -> p do f", p=P)    # [128, 8, 4096]
    w2_r = w2.rearrange("(fo p) d -> p fo d", p=P)    # [128, 32, 1024]

    # ---- startup DMAs: both HWDGE rings in parallel, ordered by deadline ----
    # sync ring: b1t (tiny), xt half 0, w2 chunk 0, xt half 1
    # scalar ring: w1 chunk 0, b2 broadcast
    nc.sync.dma_start(b1t[:], b1t_d)
    xt0 = xt_pool.tile([P, D_T, CP], mm_dt, tag="xt")
    nc.sync.dma_start(xt0[:, :, ds(0, NFREE)], xt_r[:, :, ds(0, NFREE)])
    w1c0 = w1_pool.tile([P, D_T, FC], mm_dt, tag="w1c", name="w1c")
    nc.scalar.dma_start(w1c0[:], w1_r[:, :, ds(0, FC)])
    w2c0 = w2_pool.tile([P, FC_T, D], mm_dt, tag="w2c", name="w2c")
    nc.sync.dma_start(w2c0[:], w2_r[:, ds(0, FC_T), :])
    nc.sync.dma_start(xt0[:, :, ds(NFREE, NFREE)], xt_r[:, :, ds(NFREE, NFREE)])
    nc.scalar.dma_start(b2b[:], b2[None, :].to_broadcast((P, D)))

    def load_wchunk(fci):
        w1c = w1_pool.tile([P, D_T, FC], mm_dt, tag="w1c", name="w1c")
        nc.scalar.dma_start(w1c[:], w1_r[:, :, ds(fci * FC, FC)])
        w2c = w2_pool.tile([P, FC_T, D], mm_dt, tag="w2c", name="w2c")
        nc.sync.dma_start(w2c[:], w2_r[:, ds(fci * FC_T, FC_T), :])
        return w1c, w2c

    pending = (w1c0, w2c0)

    iters = [(pss, fci) for pss in range(N_PASS) for fci in range(N_FC)]
    xts = [xt0, None]
    yaccs = [None, None]

    for it, (pss, fci) in enumerate(iters):
        w1c, w2c = pending
        if it + 1 < len(iters):
            pending = load_wchunk(iters[it + 1][1])
        if pss == 0 and fci == 3 and N_PASS == 2:
            # prefetch next pass's Xt (sync ring; lands well before pass 1)
            xt1 = xt_pool.tile([P, D_T, CP], mm_dt, tag="xt")
            nc.sync.dma_start(xt1[:, :, ds(0, CP // 2)], xt_r[:, :, ds(CP, CP // 2)])
            nc.sync.dma_start(
                xt1[:, :, ds(CP // 2, CP // 2)], xt_r[:, :, ds(CP + CP // 2, CP // 2)]
            )
            xts[1] = xt1
        xt = xts[pss]
        if fci == 0:
            yaccs[pss] = yacc_pool.tile(
                [P, CP // P, D], f32, tag="yacc", name="yacc"
            )
        yacc = yaccs[pss]
        c_base = pss * CP

        # Interleave GEMM1/GEMM2 per token half: the first half's GEMM2 runs
        # while the second xt half (or a late weight chunk) is still in
        # flight, so DMA latency hides behind PE work at startup.
        ht = ht_pool.tile([P, FC_T, CP], mm_dt, tag="ht")
        for cci in range(CP // NFREE):
            # -- GEMM1: Ht[f, c] = gelu(sum_d W1[d, f]^T X^T[d, c] + b1[f]) --
            for fti in range(FC_T):
                ps = mm_psum.tile([P, NFREE], f32, tag="mm")
                for di in range(D_T):
                    nc.tensor.matmul(
                        ps[:],
                        lhsT=w1c[:, di, ds(fti * P, P)],
                        rhs=xt[:, di, ds(cci * NFREE, NFREE)],
                        start=(di == 0),
                        stop=(di == D_T - 1),
                    )
                ft_g = fci * FC_T + fti
                nc.scalar.activation(
                    ht[:, fti, ds(cci * NFREE, NFREE)],
                    ps[:],
                    AFT.Tanh if ACT_FN == "tanh" else AFT.Gelu_apprx_tanh,
                    bias=b1t[:, ft_g : ft_g + 1],
                    scale=1.0,
                )

            # -- GEMM2: Yacc[c, d] += sum_f Ht[f, c]^T W2[f, d] --
            for ci in range(cci * (NFREE // P), (cci + 1) * (NFREE // P)):
                yb = None
                if fci == N_FC - 1:
                    yb = yb_pool.tile([P, D], f32, tag="yb", name="yb")
                for dci in range(D // NFREE):
                    ps = mm_psum.tile([P, NFREE], f32, tag="mm")
                    for fti in range(FC_T):
                        nc.tensor.matmul(
                            ps[:],
                            lhsT=ht[:, fti, ds(ci * P, P)],
                            rhs=w2c[:, fti, ds(dci * NFREE, NFREE)],
                            start=(fti == 0),
                            stop=(fti == FC_T - 1),
                        )
                    ya = yacc[:, ci, ds(dci * NFREE, NFREE)]
                    if fci == 0:
                        nc.vector.tensor_add(
                            out=ya, in0=ps[:], in1=b2b[:, ds(dci * NFREE, NFREE)]
                        )
                    elif fci == N_FC - 1:
                        # final chunk: sum lands in the bounce tile so yacc
                        # is never read by DMA (keeps yacc single-buffered,
                        # no cross-pass WAR against the writeback), and each
                        # half-row DMAs out as soon as its add completes to
                        # shorten the kernel tail.
                        nc.vector.tensor_add(
                            out=yb[:, ds(dci * NFREE, NFREE)], in0=ya, in1=ps[:]
                        )
                        nc.scalar.dma_start(
                            y[ds(c_base + ci * P, P), ds(dci * NFREE, NFREE)],
                            yb[:, ds(dci * NFREE, NFREE)],
                        )
                    else:
                        nc.vector.tensor_add(out=ya, in0=ya, in1=ps[:])


_NC_CACHE = None


def build_bass():
    global _NC_CACHE
    if _NC_CACHE is not None:
        return _NC_CACHE
    nc = bacc.Bacc("TRN2", target_bir_lowering=False, debug=False)
    f32 = mybir.dt.float32
    w_dt = mybir.dt.float32r if MM_MODE == "f32r" else f32
    xt = nc.dram_tensor("xt", [D, C], w_dt, kind="ExternalInput").ap()
    w1 = nc.dram_tensor("w1", [D, F], w_dt, kind="ExternalInput").ap()
    b1t = nc.dram_tensor("b1t", [P, F_T], f32, kind="ExternalInput").ap()
    w2 = nc.dram_tensor("w2", [F, D], w_dt, kind="ExternalInput").ap()
    b2 = nc.dram_tensor("b2", [D], f32, kind="ExternalInput").ap()
    y = nc.dram_tensor("y", [C, D], f32, kind="ExternalOutput").ap()
    with tile.TileContext(nc) as tc:
        with ExitStack() as ctx:
            _emit(ctx, tc, xt, w1, b1t, w2, b2, y)
    nc.compile()
    _NC_CACHE = nc
    return nc


def _in_maps(inputs, w1, b1, w2, b2):
    return [
        {
            "xt": np.ascontiguousarray(inputs[e * C : (e + 1) * C].T),
            "w1": np.ascontiguousarray(w1[e]),
            "b1t": np.ascontiguousarray(b1[e].reshape(F_T, P).T),
            "w2": np.ascontiguousarray(w2[e]),
            "b2": np.ascontiguousarray(b2[e]),
        }
        for e in range(E)
    ]


def kernel_run(inputs, w1, b1, w2, b2, trace=False, **trace_kwargs):
    """Run on 8 NeuronCores; returns (full_output [T, D], BassKernelResults)."""
    inputs = np.asarray(inputs, dtype=np.float32)
    w1 = np.asarray(w1, dtype=np.float32)
    b1 = np.asarray(b1, dtype=np.float32)
    w2 = np.asarray(w2, dtype=np.float32)
    b2 = np.asarray(b2, dtype=np.float32)
    nc = build_bass()
    res = run_bass_kernel_spmd(
        nc,
        _in_maps(inputs, w1, b1, w2, b2),
        core_ids=list(range(E)),
        trace=trace,
        **trace_kwargs,
    )
    out = np.concatenate([res.results[e]["y"] for e in range(E)], axis=0)
    return out, res


def kernel(inputs, w1, b1, w2, b2):
    out, _ = kernel_run(inputs, w1, b1, w2, b2, trace=False)
    return out


# revision 9
# speedup vs baseline: 1.0170x; 1.0170x over previous
"""Trainium2 Bass kernel for nn_LocalExperts (MoE expert-parallel FFN).

Reference computation (per full input):
    x  [T=16384, D=1024] -> reshape [E=8, C=2048, D]
    h  = gelu(x @ w1[e] + b1[e])     w1 [E, D, F=4096]
    y  = h @ w2[e] + b2[e]           w2 [E, F, D]
    out[T, D]

Sharding: expert parallelism across 8 NeuronCores. Expert e's tokens are
exactly rows [e*C:(e+1)*C] of the input, so core e gets that token slice
plus w1[e], b1[e], w2[e], b2[e]. No collectives needed; outputs are
concatenated on the host.

Host-side layout prep (free w.r.t. HW exec time): the token slice is
passed pre-transposed as xt [D, C] so the contraction dim D lands on
SBUF partitions via plain DMA — no PE transposes on device. b1 is
passed as b1t [128, F/128] (per-partition bias of each f-tile).

Per-core kernel (C=2048 tokens, one expert), PE runs matmuls only:
  - Two token passes of CP=1024 (xt/yacc double-buffered across passes);
    per pass, loop F in chunks of FC=512:
      GEMM1: Ht[f,c] = gelu(W1c-tiles.T @ Xt + b1)  (PSUM acc over D,
                                                     ACT drain w/ bias)
      GEMM2: Yacc[c,d] += Ht-tiles.T @ W2c          (PSUM acc over FC,
                                                     DVE acc over chunks)
  - Weight chunks stream with 1-chunk lookahead over the flat
    (pass, chunk) sequence; w1 on the scalar HWDGE ring, w2 on sync,
    so startup and steady-state loads run on both rings in parallel.
  - Matmuls run as float32r (full PE rate at N=512, ~TF32 precision,
    fp32 PSUM accumulation).
"""

import os
from contextlib import ExitStack

import numpy as np

import concourse.bass as bass
import concourse.tile as tile
from concourse import bacc
from concourse import mybir
from concourse.bass import ds, ts
from concourse.bass_utils import run_bass_kernel_spmd
from concourse.masks import make_identity

AFT = mybir.ActivationFunctionType

E = 8
D = 1024
F = 4096
T = 16384
C = T // E          # tokens per core
P = 128

N_PASS = 2          # token passes (halves SBUF residency of Xt/Yacc)
CP = C // N_PASS    # tokens per pass
FC = 512            # F chunk per iteration
NFREE = 512         # matmul moving free dim (one PSUM bank of fp32)

D_T = D // P        # 8 d-tiles
F_T = F // P        # 32 f-tiles
FC_T = FC // P      # 4 f-tiles per chunk
N_FC = F // FC      # 8 chunks per pass

# "f32r" (default): fp32 data, float32r matmul (full PE rate, ~TF32 mantissa)
# "f32": plain fp32 matmul (4 cycles/row, ~4x slower PE)
MM_MODE = os.environ.get("KERNEL_MM_MODE", "f32r")
# test-only: CoreSim lacks Gelu; "tanh" swaps the activation for sim gating
ACT_FN = os.environ.get("KERNEL_ACT", "gelu")


def _emit(ctx: ExitStack, tc: tile.TileContext, xt_d, w1, b1t_d, w2, b2, y):
    nc = tc.nc
    f32 = mybir.dt.float32
    f32r = mybir.dt.float32r
    # Matmul operand tiles live natively in this dtype; fp32r DRAM tensors
    # DMA straight into fp32r tiles (bit-identical to fp32).
    mm_dt = f32r if MM_MODE == "f32r" else f32

    consts = ctx.enter_context(tc.tile_pool(name="consts", bufs=1))
    xt_pool = ctx.enter_context(tc.tile_pool(name="xt", bufs=2))
    # yacc is only ever read by DVE (final add goes to ybounce, not back to
    # yacc), so a single buffer has no cross-pass WAR hazard with the y DMA.
    yacc_pool = ctx.enter_context(tc.tile_pool(name="yacc", bufs=1))
    yb_pool = ctx.enter_context(tc.tile_pool(name="yb", bufs=3))
    w1_pool = ctx.enter_context(tc.tile_pool(name="w1c", bufs=2))
    w2_pool = ctx.enter_context(tc.tile_pool(name="w2c", bufs=2))
    ht_pool = ctx.enter_context(tc.tile_pool(name="ht", bufs=1))
    mm_psum = ctx.enter_context(tc.tile_pool(name="mmp", bufs=8, space="PSUM"))

    identity = consts.tile([P, P], f32)
    make_identity(nc, identity[:])
    b1t = consts.tile([P, F_T], f32)
    b2b = consts.tile([P, D], f32)

    # Warm the PE HAM clock (cold 1.2GHz -> 2.4GHz needs ~3.4us of activity)
    # during the initial DMA wait, using identity matmuls.
    warm_ps = mm_psum.tile([P, NFREE], f32, tag="mm")
    for _ in range(24):
        nc.tensor.matmul(warm_ps[:, :P], lhsT=identity[:], rhs=identity[:],
                         start=True, stop=True)

    xt_r = xt_d.rearrange("(dt p) c -> p dt c", p=P)  # [128, 8, 2048]
    w1_r = w1.rearrange("(do p) f -> p do f", p=P)    # [128, 8, 4096]
    w2_r = w2.rearrange("(fo p) d -> p fo d", p=P)    # [128, 32, 1024]

    # ---- DMA ring assignment ----
    # sync ring (SP engine — FIFO has nothing but DMA triggers, so
    # deadline-critical weight chunks always fire on time): w1+w2 chunks.
    # scalar ring (ACT engine): b1t, xt, b2 broadcast, y writebacks — all
    # loose-deadline traffic that tolerates sitting behind gelu work.
    nc.sync.dma_start(b1t[:], b1t_d)
    w1c0 = w1_pool.tile([P, D_T, FC], mm_dt, tag="w1c", name="w1c")
    # first loads split in halves along dt so the first GEMM1 chain's early
    # accumulation steps start ~2us sooner
    nc.sync.dma_start(w1c0[:, ds(0, D_T // 2), :], w1_r[:, ds(0, D_T // 2), ds(0, FC)])
    nc.sync.dma_start(
        w1c0[:, ds(D_T // 2, D_T // 2), :], w1_r[:, ds(D_T // 2, D_T // 2), ds(0, FC)]
    )
    w2c0 = w2_pool.tile([P, FC_T, D], mm_dt, tag="w2c", name="w2c")
    nc.sync.dma_start(w2c0[:], w2_r[:, ds(0, FC_T), :])
    xt0 = xt_pool.tile([P, D_T, CP], mm_dt, tag="xt")
    nc.scalar.dma_start(xt0[:, ds(0, D_T // 2), ds(0, NFREE)], xt_r[:, ds(0, D_T // 2), ds(0, NFREE)])
    nc.scalar.dma_start(
        xt0[:, ds(D_T // 2, D_T // 2), ds(0, NFREE)],
        xt_r[:, ds(D_T // 2, D_T // 2), ds(0, NFREE)],
    )
    nc.scalar.dma_start(b2b[:], b2[None, :].to_broadcast((P, D)))
    nc.scalar.dma_start(xt0[:, :, ds(NFREE, NFREE)], xt_r[:, :, ds(NFREE, NFREE)])

    def load_wchunk(fci):
        w1c = w1_pool.tile([P, D_T, FC], mm_dt, tag="w1c", name="w1c")
        nc.sync.dma_start(w1c[:], w1_r[:, :, ds(fci * FC, FC)])
        w2c = w2_pool.tile([P, FC_T, D], mm_dt, tag="w2c", name="w2c")
        nc.sync.dma_start(w2c[:], w2_r[:, ds(fci * FC_T, FC_T), :])
        return w1c, w2c

    pending = (w1c0, w2c0)

    iters = [(pss, fci) for pss in range(N_PASS) for fci in range(N_FC)]
    xts = [xt0, None]
    yaccs = [None, None]

    for it, (pss, fci) in enumerate(iters):
        w1c, w2c = pending
        if it + 1 < len(iters):
            pending = load_wchunk(iters[it + 1][1])
        if pss == 0 and fci == 3 and N_PASS == 2:
            # prefetch next pass's Xt (scalar ring; lands well before pass 1)
            xt1 = xt_pool.tile([P, D_T, CP], mm_dt, tag="xt")
            nc.scalar.dma_start(xt1[:, :, ds(0, CP // 2)], xt_r[:, :, ds(CP, CP // 2)])
            nc.scalar.dma_start(
                xt1[:, :, ds(CP // 2, CP // 2)], xt_r[:, :, ds(CP + CP // 2, CP // 2)]
            )
            xts[1] = xt1
        xt = xts[pss]
        if fci == 0:
            yaccs[pss] = yacc_pool.tile(
                [P, CP // P, D], f32, tag="yacc", name="yacc"
            )
        yacc = yaccs[pss]
        c_base = pss * CP

        # Interleave GEMM1/GEMM2 per token half: the first half's GEMM2 runs
        # while the second xt half (or a late weight chunk) is still in
        # flight, so DMA latency hides behind PE work at startup.
        ht = ht_pool.tile([P, FC_T, CP], mm_dt, tag="ht")
        for cci in range(CP // NFREE):
            # -- GEMM1: Ht[f, c] = gelu(sum_d W1[d, f]^T X^T[d, c] + b1[f]) --
            for fti in range(FC_T):
                ps = mm_psum.tile([P, NFREE], f32, tag="mm")
                for di in range(D_T):
                    nc.tensor.matmul(
                        ps[:],
                        lhsT=w1c[:, di, ds(fti * P, P)],
                        rhs=xt[:, di, ds(cci * NFREE, NFREE)],
                        start=(di == 0),
                        stop=(di == D_T - 1),
                    )
                ft_g = fci * FC_T + fti
                nc.scalar.activation(
                    ht[:, fti, ds(cci * NFREE, NFREE)],
                    ps[:],
                    AFT.Tanh if ACT_FN == "tanh" else AFT.Gelu_apprx_tanh,
                    bias=b1t[:, ft_g : ft_g + 1],
                    scale=1.0,
                )

            # -- GEMM2: Yacc[c, d] += sum_f Ht[f, c]^T W2[f, d] --
            for ci in range(cci * (NFREE // P), (cci + 1) * (NFREE // P)):
                yb = None
                if fci == N_FC - 1:
                    yb = yb_pool.tile([P, D], f32, tag="yb", name="yb")
                for dci in range(D // NFREE):
                    ps = mm_psum.tile([P, NFREE], f32, tag="mm")
                    for fti in range(FC_T):
                        nc.tensor.matmul(
                            ps[:],
                            lhsT=ht[:, fti, ds(ci * P, P)],
                            rhs=w2c[:, fti, ds(dci * NFREE, NFREE)],
                            start=(fti == 0),
                            stop=(fti == FC_T - 1),
                        )
                    ya = yacc[:, ci, ds(dci * NFREE, NFREE)]
                    if fci == 0:
                        nc.vector.tensor_add(
                            out=ya, in0=ps[:], in1=b2b[:, ds(dci * NFREE, NFREE)]
                        )
                    elif fci == N_FC - 1:
                        # final chunk: sum lands in the bounce tile so yacc
                        # is never read by DMA (keeps yacc single-buffered,
                        # no cross-pass WAR against the writeback), and each
                        # half-row DMAs out as soon as its add completes to
                        # shorten the kernel tail.
                        nc.vector.tensor_add(
                            out=yb[:, ds(dci * NFREE, NFREE)], in0=ya, in1=ps[:]
                        )
                        nc.scalar.dma_start(
                            y[ds(c_base + ci * P, P), ds(dci * NFREE, NFREE)],
                            yb[:, ds(dci * NFREE, NFREE)],
                        )
                    else:
                        nc.vector.tensor_add(out=ya, in0=ya, in1=ps[:])


_NC_CACHE = None


def build_bass():
    global _NC_CACHE
    if _NC_CACHE is not None:
        return _NC_CACHE
    nc = bacc.Bacc("TRN2", target_bir_lowering=False, debug=False)
    f32 = mybir.dt.float32
    w_dt = mybir.dt.float32r if MM_MODE == "f32r" else f32
    xt = nc.dram_tensor("xt", [D, C], w_dt, kind="ExternalInput").ap()
    w1 = nc.dram_tensor("w1", [D, F], w_dt, kind="ExternalInput").ap()
    b1t = nc.dram_tensor("b1t", [P, F_T], f32, kind="ExternalInput").ap()
    w2 = nc.dram_tensor("w2", [F, D], w_dt, kind="ExternalInput").ap()
    b2 = nc.dram_tensor("b2", [D], f32, kind="ExternalInput").ap()
    y = nc.dram_tensor("y", [C, D], f32, kind="ExternalOutput").ap()
    with tile.TileContext(nc) as tc:
        with ExitStack() as ctx:
            _emit(ctx, tc, xt, w1, b1t, w2, b2, y)
    nc.compile()
    _NC_CACHE = nc
    return nc


def _in_maps(inputs, w1, b1, w2, b2):
    return [
        {
            "xt": np.ascontiguousarray(inputs[e * C : (e + 1) * C].T),
            "w1": np.ascontiguousarray(w1[e]),
            "b1t": np.ascontiguousarray(b1[e].reshape(F_T, P).T),
            "w2": np.ascontiguousarray(w2[e]),
            "b2": np.ascontiguousarray(b2[e]),
        }
        for e in range(E)
    ]


def kernel_run(inputs, w1, b1, w2, b2, trace=False, **trace_kwargs):
    """Run on 8 NeuronCores; returns (full_output [T, D], BassKernelResults)."""
    inputs = np.asarray(inputs, dtype=np.float32)
    w1 = np.asarray(w1, dtype=np.float32)
    b1 = np.asarray(b1, dtype=np.float32)
    w2 = np.asarray(w2, dtype=np.float32)
    b2 = np.asarray(b2, dtype=np.float32)
    nc = build_bass()
    res = run_bass_kernel_spmd(
        nc,
        _in_maps(inputs, w1, b1, w2, b2),
        core_ids=list(range(E)),
        trace=trace,
        **trace_kwargs,
    )
    out = np.concatenate([res.results[e]["y"] for e in range(E)], axis=0)
    return out, res


def kernel(inputs, w1, b1, w2, b2):
    out, _ = kernel_run(inputs, w1, b1, w2, b2, trace=False)
    return out


# revision 11
# speedup vs baseline: 1.0250x; 1.0078x over previous
"""Trainium2 Bass kernel for nn_LocalExperts (MoE expert-parallel FFN).

Reference computation (per full input):
    x  [T=16384, D=1024] -> reshape [E=8, C=2048, D]
    h  = gelu(x @ w1[e] + b1[e])     w1 [E, D, F=4096]
    y  = h @ w2[e] + b2[e]           w2 [E, F, D]
    out[T, D]

Sharding: expert parallelism across 8 NeuronCores. Expert e's tokens are
exactly rows [e*C:(e+1)*C] of the input, so core e gets that token slice
plus w1[e], b1[e], w2[e], b2[e]. No collectives needed; outputs are
concatenated on the host.

Host-side layout prep (free w.r.t. HW exec time): the token slice is
passed pre-transposed as xt [D, C] so the contraction dim D lands on
SBUF partitions via plain DMA — no PE transposes on device. b1 is
passed as b1t [128, F/128] (per-partition bias of each f-tile).

Per-core kernel (C=2048 tokens, one expert), PE runs matmuls only:
  - Two token passes of CP=1024 (xt/yacc double-buffered across passes);
    per pass, loop F in chunks of FC=512:
      GEMM1: Ht[f,c] = gelu(W1c-tiles.T @ Xt + b1)  (PSUM acc over D,
                                                     ACT drain w/ bias)
      GEMM2: Yacc[c,d] += Ht-tiles.T @ W2c          (PSUM acc over FC,
                                                     DVE acc over chunks)
  - Weight chunks stream with 1-chunk lookahead over the flat
    (pass, chunk) sequence; w1 on the scalar HWDGE ring, w2 on sync,
    so startup and steady-state loads run on both rings in parallel.
  - Matmuls run as float32r (full PE rate at N=512, ~TF32 precision,
    fp32 PSUM accumulation).
"""

import os
from contextlib import ExitStack

import numpy as np

import concourse.bass as bass
import concourse.tile as tile
from concourse import bacc
from concourse import mybir
from concourse.bass import ds, ts
from concourse.bass_utils import run_bass_kernel_spmd
from concourse.masks import make_identity

AFT = mybir.ActivationFunctionType

E = 8
D = 1024
F = 4096
T = 16384
C = T // E          # tokens per core
P = 128

N_PASS = 2          # token passes (halves SBUF residency of Xt/Yacc)
CP = C // N_PASS    # tokens per pass
FC = 512            # F chunk per iteration
NFREE = 512         # matmul moving free dim (one PSUM bank of fp32)

D_T = D // P        # 8 d-tiles
F_T = F // P        # 32 f-tiles
FC_T = FC // P      # 4 f-tiles per chunk
N_FC = F // FC      # 8 chunks per pass

# "f32r" (default): fp32 data, float32r matmul (full PE rate, ~TF32 mantissa)
# "f32": plain fp32 matmul (4 cycles/row, ~4x slower PE)
MM_MODE = os.environ.get("KERNEL_MM_MODE", "f32r")
# test-only: CoreSim lacks Gelu; "tanh" swaps the activation for sim gating
ACT_FN = os.environ.get("KERNEL_ACT", "gelu")


def _emit(ctx: ExitStack, tc: tile.TileContext, xt_d, w1, b1t_d, w2, b2, y):
    nc = tc.nc
    f32 = mybir.dt.float32
    f32r = mybir.dt.float32r
    # Matmul operand tiles live natively in this dtype; fp32r DRAM tensors
    # DMA straight into fp32r tiles (bit-identical to fp32).
    mm_dt = f32r if MM_MODE == "f32r" else f32

    consts = ctx.enter_context(tc.tile_pool(name="consts", bufs=1))
    xt_pool = ctx.enter_context(tc.tile_pool(name="xt", bufs=2))
    # yacc is only ever read by DVE (final add goes to ybounce, not back to
    # yacc), so a single buffer has no cross-pass WAR hazard with the y DMA.
    yacc_pool = ctx.enter_context(tc.tile_pool(name="yacc", bufs=1))
    yb_pool = ctx.enter_context(tc.tile_pool(name="yb", bufs=3))
    w1_pool = ctx.enter_context(tc.tile_pool(name="w1c", bufs=2))
    w2_pool = ctx.enter_context(tc.tile_pool(name="w2c", bufs=2))
    ht_pool = ctx.enter_context(tc.tile_pool(name="ht", bufs=1))
    mm_psum = ctx.enter_context(tc.tile_pool(name="mmp", bufs=8, space="PSUM"))

    # warmup operand — contents irrelevant, DVE memset is ready ~instantly
    # (make_identity's GPSIMD path takes ~2.5us to start)
    identity = consts.tile([P, P], f32)
    nc.vector.memset(identity[:], 1.0)
    b1t = consts.tile([P, F_T], f32)
    b2b = consts.tile([P, D], f32)

    # Warm the PE HAM clock (cold 1.2GHz -> 2.4GHz needs ~3.4us of activity)
    # during the initial DMA wait, using identity matmuls.
    warm_ps = mm_psum.tile([P, NFREE], f32, tag="mm")
    for _ in range(24):
        nc.tensor.matmul(warm_ps[:, :P], lhsT=identity[:], rhs=identity[:],
                         start=True, stop=True)

    xt_r = xt_d.rearrange("(dt p) c -> p dt c", p=P)  # [128, 8, 2048]
    w1_r = w1.rearrange("(do p) f -> p do f", p=P)    # [128, 8, 4096]
    w2_r = w2.rearrange("(fo p) d -> p fo d", p=P)    # [128, 32, 1024]

    # ---- DMA ring assignment ----
    # sync ring (SP engine — FIFO has nothing but DMA triggers, so
    # deadline-critical weight chunks always fire on time): w1+w2 chunks.
    # scalar ring (ACT engine): b1t, xt, b2 broadcast, y writebacks — all
    # loose-deadline traffic that tolerates sitting behind gelu work.
    nc.sync.dma_start(b1t[:], b1t_d)
    w1c0 = w1_pool.tile([P, D_T, FC], mm_dt, tag="w1c", name="w1c")
    # first loads split in halves along dt so the first GEMM1 chain's early
    # accumulation steps start ~2us sooner
    nc.sync.dma_start(w1c0[:, ds(0, D_T // 2), :], w1_r[:, ds(0, D_T // 2), ds(0, FC)])
    nc.sync.dma_start(
        w1c0[:, ds(D_T // 2, D_T // 2), :], w1_r[:, ds(D_T // 2, D_T // 2), ds(0, FC)]
    )
    w2c0 = w2_pool.tile([P, FC_T, D], mm_dt, tag="w2c", name="w2c")
    nc.sync.dma_start(w2c0[:], w2_r[:, ds(0, FC_T), :])
    xt0 = xt_pool.tile([P, D_T, CP], mm_dt, tag="xt")
    nc.scalar.dma_start(xt0[:, ds(0, D_T // 2), ds(0, NFREE)], xt_r[:, ds(0, D_T // 2), ds(0, NFREE)])
    nc.scalar.dma_start(
        xt0[:, ds(D_T // 2, D_T // 2), ds(0, NFREE)],
        xt_r[:, ds(D_T // 2, D_T // 2), ds(0, NFREE)],
    )
    nc.scalar.dma_start(b2b[:], b2[None, :].to_broadcast((P, D)))
    nc.scalar.dma_start(xt0[:, :, ds(NFREE, NFREE)], xt_r[:, :, ds(NFREE, NFREE)])

    def load_wchunk(fci):
        w1c = w1_pool.tile([P, D_T, FC], mm_dt, tag="w1c", name="w1c")
        nc.sync.dma_start(w1c[:], w1_r[:, :, ds(fci * FC, FC)])
        w2c = w2_pool.tile([P, FC_T, D], mm_dt, tag="w2c", name="w2c")
        nc.sync.dma_start(w2c[:], w2_r[:, ds(fci * FC_T, FC_T), :])
        return w1c, w2c

    pending = (w1c0, w2c0)

    iters = [(pss, fci) for pss in range(N_PASS) for fci in range(N_FC)]
    xts = [xt0, None]
    yaccs = [None, None]

    for it, (pss, fci) in enumerate(iters):
        w1c, w2c = pending
        if it + 1 < len(iters):
            pending = load_wchunk(iters[it + 1][1])
        if pss == 0 and fci == 3 and N_PASS == 2:
            # prefetch next pass's Xt (scalar ring; lands well before pass 1)
            xt1 = xt_pool.tile([P, D_T, CP], mm_dt, tag="xt")
            nc.scalar.dma_start(xt1[:, :, ds(0, CP // 2)], xt_r[:, :, ds(CP, CP // 2)])
            nc.scalar.dma_start(
                xt1[:, :, ds(CP // 2, CP // 2)], xt_r[:, :, ds(CP + CP // 2, CP // 2)]
            )
            xts[1] = xt1
        xt = xts[pss]
        if fci == 0:
            yaccs[pss] = yacc_pool.tile(
                [P, CP // P, D], f32, tag="yacc", name="yacc"
            )
        yacc = yaccs[pss]
        c_base = pss * CP

        # ---- GEMM1: Ht[f, c] = gelu(sum_d W1[d, f]^T X^T[d, c] + b1[f]) ----
        ht = ht_pool.tile([P, FC_T, CP], mm_dt, tag="ht")
        for cci in range(CP // NFREE):
            for fti in range(FC_T):
                ps = mm_psum.tile([P, NFREE], f32, tag="mm")
                for di in range(D_T):
                    nc.tensor.matmul(
                        ps[:],
                        lhsT=w1c[:, di, ds(fti * P, P)],
                        rhs=xt[:, di, ds(cci * NFREE, NFREE)],
                        start=(di == 0),
                        stop=(di == D_T - 1),
                    )
                ft_g = fci * FC_T + fti
                nc.scalar.activation(
                    ht[:, fti, ds(cci * NFREE, NFREE)],
                    ps[:],
                    AFT.Tanh if ACT_FN == "tanh" else AFT.Gelu_apprx_tanh,
                    bias=b1t[:, ft_g : ft_g + 1],
                    scale=1.0,
                )

        # ---- GEMM2: Yacc[c, d] += sum_f Ht[f, c]^T W2[f, d] ----
        for ci in range(CP // P):
            yb = None
            if fci == N_FC - 1:
                yb = yb_pool.tile([P, D], f32, tag="yb", name="yb")
            for dci in range(D // NFREE):
                ps = mm_psum.tile([P, NFREE], f32, tag="mm")
                for fti in range(FC_T):
                    nc.tensor.matmul(
                        ps[:],
                        lhsT=ht[:, fti, ds(ci * P, P)],
                        rhs=w2c[:, fti, ds(dci * NFREE, NFREE)],
                        start=(fti == 0),
                        stop=(fti == FC_T - 1),
                    )
                ya = yacc[:, ci, ds(dci * NFREE, NFREE)]
                if fci == 0:
                    nc.vector.tensor_add(
                        out=ya, in0=ps[:], in1=b2b[:, ds(dci * NFREE, NFREE)]
                    )
                elif fci == N_FC - 1:
                    # final chunk: sum lands in the bounce tile so yacc is
                    # never read by DMA (keeps yacc single-buffered, no
                    # cross-pass WAR against the writeback), and each
                    # half-row DMAs out as soon as its add completes to
                    # shorten the kernel tail.
                    nc.vector.tensor_add(
                        out=yb[:, ds(dci * NFREE, NFREE)], in0=ya, in1=ps[:]
                    )
                    nc.scalar.dma_start(
                        y[ds(c_base + ci * P, P), ds(dci * NFREE, NFREE)],
                        yb[:, ds(dci * NFREE, NFREE)],
                    )
                else:
                    nc.vector.tensor_add(out=ya, in0=ya, in1=ps[:])


_NC_CACHE = None


def build_bass():
    global _NC_CACHE
    if _NC_CACHE is not None:
        return _NC_CACHE
    nc = bacc.Bacc("TRN2", target_bir_lowering=False, debug=False)
    f32 = mybir.dt.float32
    w_dt = mybir.dt.float32r if MM_MODE == "f32r" else f32
    xt = nc.dram_tensor("xt", [D, C], w_dt, kind="ExternalInput").ap()
    w1 = nc.dram_tensor("w1", [D, F], w_dt, kind="ExternalInput").ap()
    b1t = nc.dram_tensor("b1t", [P, F_T], f32, kind="ExternalInput").ap()
    w2 = nc.dram_tensor("w2", [F, D], w_dt, kind="ExternalInput").ap()
    b2 = nc.dram_tensor("b2", [D], f32, kind="ExternalInput").ap()
    y = nc.dram_tensor("y", [C, D], f32, kind="ExternalOutput").ap()
    with tile.TileContext(nc) as tc:
        with ExitStack() as ctx:
            _emit(ctx, tc, xt, w1, b1t, w2, b2, y)
    nc.compile()
    _NC_CACHE = nc
    return nc


def _in_maps(inputs, w1, b1, w2, b2):
    return [
        {
            "xt": np.ascontiguousarray(inputs[e * C : (e + 1) * C].T),
            "w1": np.ascontiguousarray(w1[e]),
            "b1t": np.ascontiguousarray(b1[e].reshape(F_T, P).T),
            "w2": np.ascontiguousarray(w2[e]),
            "b2": np.ascontiguousarray(b2[e]),
        }
        for e in range(E)
    ]


def kernel_run(inputs, w1, b1, w2, b2, trace=False, **trace_kwargs):
    """Run on 8 NeuronCores; returns (full_output [T, D], BassKernelResults)."""
    inputs = np.asarray(inputs, dtype=np.float32)
    w1 = np.asarray(w1, dtype=np.float32)
    b1 = np.asarray(b1, dtype=np.float32)
    w2 = np.asarray(w2, dtype=np.float32)
    b2 = np.asarray(b2, dtype=np.float32)
    nc = build_bass()
    res = run_bass_kernel_spmd(
        nc,
        _in_maps(inputs, w1, b1, w2, b2),
        core_ids=list(range(E)),
        trace=trace,
        **trace_kwargs,
    )
    out = np.concatenate([res.results[e]["y"] for e in range(E)], axis=0)
    return out, res


def kernel(inputs, w1, b1, w2, b2):
    out, _ = kernel_run(inputs, w1, b1, w2, b2, trace=False)
    return out


# revision 13
# speedup vs baseline: 1.0298x; 1.0047x over previous
"""Trainium2 Bass kernel for nn_LocalExperts (MoE expert-parallel FFN).

Reference computation (per full input):
    x  [T=16384, D=1024] -> reshape [E=8, C=2048, D]
    h  = gelu(x @ w1[e] + b1[e])     w1 [E, D, F=4096]
    y  = h @ w2[e] + b2[e]           w2 [E, F, D]
    out[T, D]

Sharding: expert parallelism across 8 NeuronCores. Expert e's tokens are
exactly rows [e*C:(e+1)*C] of the input, so core e gets that token slice
plus w1[e], b1[e], w2[e], b2[e]. No collectives needed; outputs are
concatenated on the host.

Host-side layout prep (free w.r.t. HW exec time): the token slice is
passed pre-transposed as xt [D, C] so the contraction dim D lands on
SBUF partitions via plain DMA — no PE transposes on device. b1 is
passed as b1t [128, F/128] (per-partition bias of each f-tile).

Per-core kernel (C=2048 tokens, one expert), PE runs matmuls only:
  - Two token passes of CP=1024 (xt/yacc double-buffered across passes);
    per pass, loop F in chunks of FC=512:
      GEMM1: Ht[f,c] = gelu(W1c-tiles.T @ Xt + b1)  (PSUM acc over D,
                                                     ACT drain w/ bias)
      GEMM2: Yacc[c,d] += Ht-tiles.T @ W2c          (PSUM acc over FC,
                                                     DVE acc over chunks)
  - Weight chunks stream with 1-chunk lookahead over the flat
    (pass, chunk) sequence; w1 on the scalar HWDGE ring, w2 on sync,
    so startup and steady-state loads run on both rings in parallel.
  - Matmuls run as float32r (full PE rate at N=512, ~TF32 precision,
    fp32 PSUM accumulation).
"""

import os
from contextlib import ExitStack

import numpy as np

import concourse.bass as bass
import concourse.tile as tile
from concourse import bacc
from concourse import mybir
from concourse.bass import ds, ts
from concourse.bass_utils import run_bass_kernel_spmd
from concourse.masks import make_identity

AFT = mybir.ActivationFunctionType

E = 8
D = 1024
F = 4096
T = 16384
C = T // E          # tokens per core
P = 128

N_PASS = 2          # token passes (halves SBUF residency of Xt/Yacc)
CP = C // N_PASS    # tokens per pass
FC = 512            # F chunk per iteration
NFREE = 512         # matmul moving free dim (one PSUM bank of fp32)

D_T = D // P        # 8 d-tiles
F_T = F // P        # 32 f-tiles
FC_T = FC // P      # 4 f-tiles per chunk
N_FC = F // FC      # 8 chunks per pass

# "f32r" (default): fp32 data, float32r matmul (full PE rate, ~TF32 mantissa)
# "f32": plain fp32 matmul (4 cycles/row, ~4x slower PE)
MM_MODE = os.environ.get("KERNEL_MM_MODE", "f32r")
# test-only: CoreSim lacks Gelu; "tanh" swaps the activation for sim gating
ACT_FN = os.environ.get("KERNEL_ACT", "gelu")


def _emit(ctx: ExitStack, tc: tile.TileContext, xt_d, w1, b1t_d, w2, b2, y):
    nc = tc.nc
    f32 = mybir.dt.float32
    f32r = mybir.dt.float32r
    # Matmul operand tiles live natively in this dtype; fp32r DRAM tensors
    # DMA straight into fp32r tiles (bit-identical to fp32).
    mm_dt = f32r if MM_MODE == "f32r" else f32

    consts = ctx.enter_context(tc.tile_pool(name="consts", bufs=1))
    xt_pool = ctx.enter_context(tc.tile_pool(name="xt", bufs=2))
    # yacc is only ever read by DVE (final add goes to ybounce, not back to
    # yacc), so a single buffer has no cross-pass WAR hazard with the y DMA.
    yacc_pool = ctx.enter_context(tc.tile_pool(name="yacc", bufs=1))
    yb_pool = ctx.enter_context(tc.tile_pool(name="yb", bufs=3))
    w1_pool = ctx.enter_context(tc.tile_pool(name="w1c", bufs=2))
    w2_pool = ctx.enter_context(tc.tile_pool(name="w2c", bufs=2))
    ht_pool = ctx.enter_context(tc.tile_pool(name="ht", bufs=1))
    mm_psum = ctx.enter_context(tc.tile_pool(name="mmp", bufs=8, space="PSUM"))

    # warmup operand — contents irrelevant; GPSIMD make_identity is ready
    # ~3us in, so the warmup matmuls finish before real operands land and
    # never delay the real stream in the PE FIFO
    identity = consts.tile([P, P], f32)
    make_identity(nc, identity[:])
    b1t = consts.tile([P, F_T], f32)
    b2b = consts.tile([P, D], f32)

    # Warm the PE HAM clock (cold 1.2GHz -> 2.4GHz needs ~3.4us of activity)
    # during the initial DMA wait, using identity matmuls.
    warm_ps = mm_psum.tile([P, NFREE], f32, tag="mm")
    for _ in range(24):
        nc.tensor.matmul(warm_ps[:, :P], lhsT=identity[:], rhs=identity[:],
                         start=True, stop=True)

    xt_r = xt_d.rearrange("(dt p) c -> p dt c", p=P)  # [128, 8, 2048]
    w1_r = w1.rearrange("(do p) f -> p do f", p=P)    # [128, 8, 4096]
    w2_r = w2.rearrange("(fo p) d -> p fo d", p=P)    # [128, 32, 1024]

    # ---- DMA ring assignment ----
    # sync ring (SP engine — FIFO has nothing but DMA triggers, so
    # deadline-critical weight chunks always fire on time): w1+w2 chunks.
    # scalar ring (ACT engine): b1t, xt, b2 broadcast, y writebacks — all
    # loose-deadline traffic that tolerates sitting behind gelu work.
    nc.sync.dma_start(b1t[:], b1t_d)
    w1c0 = w1_pool.tile([P, D_T, FC], mm_dt, tag="w1c", name="w1c")
    # first loads split in halves along dt so the first GEMM1 chain's early
    # accumulation steps start ~2us sooner
    nc.sync.dma_start(w1c0[:, ds(0, D_T // 2), :], w1_r[:, ds(0, D_T // 2), ds(0, FC)])
    nc.sync.dma_start(
        w1c0[:, ds(D_T // 2, D_T // 2), :], w1_r[:, ds(D_T // 2, D_T // 2), ds(0, FC)]
    )
    w2c0 = w2_pool.tile([P, FC_T, D], mm_dt, tag="w2c", name="w2c")
    nc.sync.dma_start(w2c0[:], w2_r[:, ds(0, FC_T), :])
    # xt halves land in consumption order (cci=0 needs dt-halves of the
    # first 512 tokens first); b2b is only needed by the first GEMM2 add
    xt0 = xt_pool.tile([P, D_T, CP], mm_dt, tag="xt")
    for h in range(2):
        for dh in range(2):
            nc.scalar.dma_start(
                xt0[:, ds(dh * (D_T // 2), D_T // 2), ds(h * NFREE, NFREE)],
                xt_r[:, ds(dh * (D_T // 2), D_T // 2), ds(h * NFREE, NFREE)],
            )
    nc.scalar.dma_start(b2b[:], b2[None, :].to_broadcast((P, D)))

    def load_wchunk(fci):
        w1c = w1_pool.tile([P, D_T, FC], mm_dt, tag="w1c", name="w1c")
        nc.sync.dma_start(w1c[:], w1_r[:, :, ds(fci * FC, FC)])
        w2c = w2_pool.tile([P, FC_T, D], mm_dt, tag="w2c", name="w2c")
        nc.sync.dma_start(w2c[:], w2_r[:, ds(fci * FC_T, FC_T), :])
        return w1c, w2c

    pending = (w1c0, w2c0)

    iters = [(pss, fci) for pss in range(N_PASS) for fci in range(N_FC)]
    xts = [xt0, None]
    yaccs = [None, None]

    for it, (pss, fci) in enumerate(iters):
        w1c, w2c = pending
        if it + 1 < len(iters):
            pending = load_wchunk(iters[it + 1][1])
        if pss == 0 and fci == 3 and N_PASS == 2:
            # prefetch next pass's Xt (scalar ring; lands well before pass 1)
            xt1 = xt_pool.tile([P, D_T, CP], mm_dt, tag="xt")
            nc.scalar.dma_start(xt1[:, :, ds(0, CP // 2)], xt_r[:, :, ds(CP, CP // 2)])
            nc.scalar.dma_start(
                xt1[:, :, ds(CP // 2, CP // 2)], xt_r[:, :, ds(CP + CP // 2, CP // 2)]
            )
            xts[1] = xt1
        xt = xts[pss]
        if fci == 0:
            yaccs[pss] = yacc_pool.tile(
                [P, CP // P, D], f32, tag="yacc", name="yacc"
            )
        yacc = yaccs[pss]
        c_base = pss * CP

        # ---- GEMM1: Ht[f, c] = gelu(sum_d W1[d, f]^T X^T[d, c] + b1[f]) ----
        ht = ht_pool.tile([P, FC_T, CP], mm_dt, tag="ht")
        for cci in range(CP // NFREE):
            for fti in range(FC_T):
                ps = mm_psum.tile([P, NFREE], f32, tag="mm")
                for di in range(D_T):
                    nc.tensor.matmul(
                        ps[:],
                        lhsT=w1c[:, di, ds(fti * P, P)],
                        rhs=xt[:, di, ds(cci * NFREE, NFREE)],
                        start=(di == 0),
                        stop=(di == D_T - 1),
                    )
                ft_g = fci * FC_T + fti
                nc.scalar.activation(
                    ht[:, fti, ds(cci * NFREE, NFREE)],
                    ps[:],
                    AFT.Tanh if ACT_FN == "tanh" else AFT.Gelu_apprx_tanh,
                    bias=b1t[:, ft_g : ft_g + 1],
                    scale=1.0,
                )

        # ---- GEMM2: Yacc[c, d] += sum_f Ht[f, c]^T W2[f, d] ----
        for ci in range(CP // P):
            yb = None
            if fci == N_FC - 1:
                yb = yb_pool.tile([P, D], f32, tag="yb", name="yb")
            for dci in range(D // NFREE):
                ps = mm_psum.tile([P, NFREE], f32, tag="mm")
                for fti in range(FC_T):
                    nc.tensor.matmul(
                        ps[:],
                        lhsT=ht[:, fti, ds(ci * P, P)],
                        rhs=w2c[:, fti, ds(dci * NFREE, NFREE)],
                        start=(fti == 0),
                        stop=(fti == FC_T - 1),
                    )
                ya = yacc[:, ci, ds(dci * NFREE, NFREE)]
                if fci == 0:
                    nc.vector.tensor_add(
                        out=ya, in0=ps[:], in1=b2b[:, ds(dci * NFREE, NFREE)]
                    )
                elif fci == N_FC - 1:
                    # final chunk: sum lands in the bounce tile so yacc is
                    # never read by DMA (keeps yacc single-buffered, no
                    # cross-pass WAR against the writeback), and each
                    # half-row DMAs out as soon as its add completes to
                    # shorten the kernel tail.
                    nc.vector.tensor_add(
                        out=yb[:, ds(dci * NFREE, NFREE)], in0=ya, in1=ps[:]
                    )
                    nc.scalar.dma_start(
                        y[ds(c_base + ci * P, P), ds(dci * NFREE, NFREE)],
                        yb[:, ds(dci * NFREE, NFREE)],
                    )
                else:
                    nc.vector.tensor_add(out=ya, in0=ya, in1=ps[:])


_NC_CACHE = None


def build_bass():
    global _NC_CACHE
    if _NC_CACHE is not None:
        return _NC_CACHE
    nc = bacc.Bacc("TRN2", target_bir_lowering=False, debug=False)
    f32 = mybir.dt.float32
    w_dt = mybir.dt.float32r if MM_MODE == "f32r" else f32
    xt = nc.dram_tensor("xt", [D, C], w_dt, kind="ExternalInput").ap()
    w1 = nc.dram_tensor("w1", [D, F], w_dt, kind="ExternalInput").ap()
    b1t = nc.dram_tensor("b1t", [P, F_T], f32, kind="ExternalInput").ap()
    w2 = nc.dram_tensor("w2", [F, D], w_dt, kind="ExternalInput").ap()
    b2 = nc.dram_tensor("b2", [D], f32, kind="ExternalInput").ap()
    y = nc.dram_tensor("y", [C, D], f32, kind="ExternalOutput").ap()
    with tile.TileContext(nc) as tc:
        with ExitStack() as ctx:
            _emit(ctx, tc, xt, w1, b1t, w2, b2, y)
    nc.compile()
    _NC_CACHE = nc
    return nc


def _in_maps(inputs, w1, b1, w2, b2):
    return [
        {
            "xt": np.ascontiguousarray(inputs[e * C : (e + 1) * C].T),
            "w1": np.ascontiguousarray(w1[e]),
            "b1t": np.ascontiguousarray(b1[e].reshape(F_T, P).T),
            "w2": np.ascontiguousarray(w2[e]),
            "b2": np.ascontiguousarray(b2[e]),
        }
        for e in range(E)
    ]


def kernel_run(inputs, w1, b1, w2, b2, trace=False, **trace_kwargs):
    """Run on 8 NeuronCores; returns (full_output [T, D], BassKernelResults)."""
    inputs = np.asarray(inputs, dtype=np.float32)
    w1 = np.asarray(w1, dtype=np.float32)
    b1 = np.asarray(b1, dtype=np.float32)
    w2 = np.asarray(w2, dtype=np.float32)
    b2 = np.asarray(b2, dtype=np.float32)
    nc = build_bass()
    res = run_bass_kernel_spmd(
        nc,
        _in_maps(inputs, w1, b1, w2, b2),
        core_ids=list(range(E)),
        trace=trace,
        **trace_kwargs,
    )
    out = np.concatenate([res.results[e]["y"] for e in range(E)], axis=0)
    return out, res


def kernel(inputs, w1, b1, w2, b2):
    out, _ = kernel_run(inputs, w1, b1, w2, b2, trace=False)
    return out


# revision 15
# speedup vs baseline: 1.1075x; 1.0755x over previous
"""Trainium2 Bass kernel for nn_LocalExperts (MoE expert-parallel FFN).

Reference computation (per full input):
    x  [T=16384, D=1024] -> reshape [E=8, C=2048, D]
    h  = gelu(x @ w1[e] + b1[e])     w1 [E, D, F=4096]
    y  = h @ w2[e] + b2[e]           w2 [E, F, D]
    out[T, D]

Sharding: expert parallelism across 8 NeuronCores. Expert e's tokens are
exactly rows [e*C:(e+1)*C] of the input, so core e gets that token slice
plus w1[e], b1[e], w2[e], b2[e]. No collectives needed; outputs are
concatenated on the host.

Host-side layout prep (free w.r.t. HW exec time): the token slice is
passed pre-transposed as xt [D, C] so the contraction dim D lands on
SBUF partitions via plain DMA — no PE transposes on device. b1 is
passed as b1t [128, F/128] (per-partition bias of each f-tile).

Numerics: matmul operands are bf16 (measured end-to-end rel-l2 vs the
fp32 reference ~3.2e-3; PSUM accumulation stays fp32, biases and the
output stay fp32). bf16 halves all weight/activation DMA, which lets
the whole working set stream in ONE token pass (weights fetched once)
with ample SBUF headroom, and enables the PE's fast-weight-load path.

Per-core kernel (C=2048 tokens, one expert), PE runs matmuls only:
  - Loop F in chunks of FC=512:
      GEMM1: Ht[f,c] = gelu(W1c-tiles.T @ Xt + b1)  (PSUM acc over D,
                                                     ACT drain w/ bias)
      GEMM2: Yacc[c,d] += Ht-tiles.T @ W2c          (PSUM acc over FC,
                                                     DVE acc over chunks)
  - Weight chunks stream with 1-chunk lookahead on the sync ring (SP
    engine: nothing but DMA triggers, so deadlines always fire); xt,
    b2 broadcast and y writebacks ride the scalar ring.
  - Final chunk sums land in small bounce tiles and DMA out per
    half-row immediately (short kernel tail; yacc is never read by
    DMA).
"""

import os
from contextlib import ExitStack

import numpy as np
import ml_dtypes

import concourse.bass as bass
import concourse.tile as tile
from concourse import bacc
from concourse import mybir
from concourse.bass import ds, ts
from concourse.bass_utils import run_bass_kernel_spmd
from concourse.masks import make_identity

AFT = mybir.ActivationFunctionType

E = 8
D = 1024
F = 4096
T = 16384
C = T // E          # tokens per core
P = 128

FC = 512            # F chunk per iteration
NFREE = 512         # matmul moving free dim (one PSUM bank of fp32)

D_T = D // P        # 8 d-tiles
F_T = F // P        # 32 f-tiles
FC_T = FC // P      # 4 f-tiles per chunk
N_FC = F // FC      # 8 chunks

# test-only: CoreSim lacks Gelu; "tanh" swaps the activation for sim gating
ACT_FN = os.environ.get("KERNEL_ACT", "gelu")
MM_MODE = "bf16"  # informational (test.py prints it)


def _emit(ctx: ExitStack, tc: tile.TileContext, xt_d, w1, b1t_d, w2, b2, y):
    nc = tc.nc
    f32 = mybir.dt.float32
    bf16 = mybir.dt.bfloat16

    consts = ctx.enter_context(tc.tile_pool(name="consts", bufs=1))
    xt_pool = ctx.enter_context(tc.tile_pool(name="xt", bufs=1))
    yacc_pool = ctx.enter_context(tc.tile_pool(name="yacc", bufs=1))
    yb_pool = ctx.enter_context(tc.tile_pool(name="yb", bufs=3))
    w1_pool = ctx.enter_context(tc.tile_pool(name="w1c", bufs=2))
    w2_pool = ctx.enter_context(tc.tile_pool(name="w2c", bufs=2))
    ht_pool = ctx.enter_context(tc.tile_pool(name="ht", bufs=1))
    mm_psum = ctx.enter_context(tc.tile_pool(name="mmp", bufs=8, space="PSUM"))

    # warmup operand — contents irrelevant; GPSIMD make_identity is ready
    # ~3us in, so the warmup matmuls finish before real operands land and
    # never delay the real stream in the PE FIFO
    identity = consts.tile([P, P], bf16)
    make_identity(nc, identity[:])
    b1t = consts.tile([P, F_T], f32)
    b2b = consts.tile([P, D], f32)

    # Warm the PE HAM clock (cold 1.2GHz -> 2.4GHz needs ~3.4us of activity)
    # during the initial DMA wait.
    warm_ps = mm_psum.tile([P, NFREE], f32, tag="mm")
    for _ in range(24):
        nc.tensor.matmul(warm_ps[:, :P], lhsT=identity[:], rhs=identity[:],
                         start=True, stop=True)

    xt_r = xt_d.rearrange("(dt p) c -> p dt c", p=P)  # [128, 8, 2048]
    w1_r = w1.rearrange("(do p) f -> p do f", p=P)    # [128, 8, 4096]
    w2_r = w2.rearrange("(fo p) d -> p fo d", p=P)    # [128, 32, 1024]

    # ---- startup DMAs, ordered by consumption deadline ----
    # sync ring: b1t (tiny), w1 chunk 0 in dt-halves, w2 chunk 0
    nc.sync.dma_start(b1t[:], b1t_d)
    w1c0 = w1_pool.tile([P, D_T, FC], bf16, tag="w1c", name="w1c")
    nc.sync.dma_start(w1c0[:, ds(0, D_T // 2), :], w1_r[:, ds(0, D_T // 2), ds(0, FC)])
    nc.sync.dma_start(
        w1c0[:, ds(D_T // 2, D_T // 2), :], w1_r[:, ds(D_T // 2, D_T // 2), ds(0, FC)]
    )
    w2c0 = w2_pool.tile([P, FC_T, D], bf16, tag="w2c", name="w2c")
    nc.sync.dma_start(w2c0[:], w2_r[:, ds(0, FC_T), :])
    # scalar ring: xt in cci-block order (first block split in dt-halves so
    # the first GEMM1 chain starts earliest), then the b2 broadcast
    xt = xt_pool.tile([P, D_T, C], bf16, tag="xt")
    nc.scalar.dma_start(
        xt[:, ds(0, D_T // 2), ds(0, NFREE)], xt_r[:, ds(0, D_T // 2), ds(0, NFREE)]
    )
    nc.scalar.dma_start(
        xt[:, ds(D_T // 2, D_T // 2), ds(0, NFREE)],
        xt_r[:, ds(D_T // 2, D_T // 2), ds(0, NFREE)],
    )
    for cci in range(1, C // NFREE):
        nc.scalar.dma_start(
            xt[:, :, ds(cci * NFREE, NFREE)], xt_r[:, :, ds(cci * NFREE, NFREE)]
        )
    nc.scalar.dma_start(b2b[:], b2[None, :].to_broadcast((P, D)))

    def load_wchunk(fci):
        w1c = w1_pool.tile([P, D_T, FC], bf16, tag="w1c", name="w1c")
        nc.sync.dma_start(w1c[:], w1_r[:, :, ds(fci * FC, FC)])
        w2c = w2_pool.tile([P, FC_T, D], bf16, tag="w2c", name="w2c")
        nc.sync.dma_start(w2c[:], w2_r[:, ds(fci * FC_T, FC_T), :])
        return w1c, w2c

    pending = (w1c0, w2c0)
    yacc = yacc_pool.tile([P, C // P, D], f32, tag="yacc")

    for fci in range(N_FC):
        w1c, w2c = pending
        if fci + 1 < N_FC:
            pending = load_wchunk(fci + 1)

        # ---- GEMM1: Ht[f, c] = gelu(sum_d W1[d, f]^T X^T[d, c] + b1[f]) ----
        ht = ht_pool.tile([P, FC_T, C], bf16, tag="ht")
        for cci in range(C // NFREE):
            for fti in range(FC_T):
                ps = mm_psum.tile([P, NFREE], f32, tag="mm")
                for di in range(D_T):
                    nc.tensor.matmul(
                        ps[:],
                        lhsT=w1c[:, di, ds(fti * P, P)],
                        rhs=xt[:, di, ds(cci * NFREE, NFREE)],
                        start=(di == 0),
                        stop=(di == D_T - 1),
                    )
                ft_g = fci * FC_T + fti
                nc.scalar.activation(
                    ht[:, fti, ds(cci * NFREE, NFREE)],
                    ps[:],
                    AFT.Tanh if ACT_FN == "tanh" else AFT.Gelu_apprx_tanh,
                    bias=b1t[:, ft_g : ft_g + 1],
                    scale=1.0,
                )

        # ---- GEMM2: Yacc[c, d] += sum_f Ht[f, c]^T W2[f, d] ----
        for ci in range(C // P):
            yb = None
            if fci == N_FC - 1:
                yb = yb_pool.tile([P, D], f32, tag="yb", name="yb")
            for dci in range(D // NFREE):
                ps = mm_psum.tile([P, NFREE], f32, tag="mm")
                for fti in range(FC_T):
                    nc.tensor.matmul(
                        ps[:],
                        lhsT=ht[:, fti, ds(ci * P, P)],
                        rhs=w2c[:, fti, ds(dci * NFREE, NFREE)],
                        start=(fti == 0),
                        stop=(fti == FC_T - 1),
                    )
                ya = yacc[:, ci, ds(dci * NFREE, NFREE)]
                if fci == 0:
                    nc.vector.tensor_add(
                        out=ya, in0=ps[:], in1=b2b[:, ds(dci * NFREE, NFREE)]
                    )
                elif fci == N_FC - 1:
                    # final chunk: sum lands in the bounce tile so yacc is
                    # never read by DMA, and each half-row DMAs out as soon
                    # as its add completes to shorten the kernel tail.
                    nc.vector.tensor_add(
                        out=yb[:, ds(dci * NFREE, NFREE)], in0=ya, in1=ps[:]
                    )
                    nc.scalar.dma_start(
                        y[ds(ci * P, P), ds(dci * NFREE, NFREE)],
                        yb[:, ds(dci * NFREE, NFREE)],
                    )
                else:
                    nc.vector.tensor_add(out=ya, in0=ya, in1=ps[:])


_NC_CACHE = None


def build_bass():
    global _NC_CACHE
    if _NC_CACHE is not None:
        return _NC_CACHE
    nc = bacc.Bacc("TRN2", target_bir_lowering=False, debug=False)
    f32 = mybir.dt.float32
    bf16 = mybir.dt.bfloat16
    xt = nc.dram_tensor("xt", [D, C], bf16, kind="ExternalInput").ap()
    w1 = nc.dram_tensor("w1", [D, F], bf16, kind="ExternalInput").ap()
    b1t = nc.dram_tensor("b1t", [P, F_T], f32, kind="ExternalInput").ap()
    w2 = nc.dram_tensor("w2", [F, D], bf16, kind="ExternalInput").ap()
    b2 = nc.dram_tensor("b2", [D], f32, kind="ExternalInput").ap()
    y = nc.dram_tensor("y", [C, D], f32, kind="ExternalOutput").ap()
    with tile.TileContext(nc) as tc:
        with ExitStack() as ctx:
            _emit(ctx, tc, xt, w1, b1t, w2, b2, y)
    nc.compile()
    _NC_CACHE = nc
    return nc


def _in_maps(inputs, w1, b1, w2, b2):
    bf = ml_dtypes.bfloat16
    return [
        {
            "xt": np.ascontiguousarray(inputs[e * C : (e + 1) * C].T).astype(bf),
            "w1": np.ascontiguousarray(w1[e]).astype(bf),
            "b1t": np.ascontiguousarray(b1[e].reshape(F_T, P).T),
            "w2": np.ascontiguousarray(w2[e]).astype(bf),
            "b2": np.ascontiguousarray(b2[e]),
        }
        for e in range(E)
    ]


def kernel_run(inputs, w1, b1, w2, b2, trace=False, **trace_kwargs):
    """Run on 8 NeuronCores; returns (full_output [T, D], BassKernelResults)."""
    inputs = np.asarray(inputs, dtype=np.float32)
    w1 = np.asarray(w1, dtype=np.float32)
    b1 = np.asarray(b1, dtype=np.float32)
    w2 = np.asarray(w2, dtype=np.float32)
    b2 = np.asarray(b2, dtype=np.float32)
    nc = build_bass()
    res = run_bass_kernel_spmd(
        nc,
        _in_maps(inputs, w1, b1, w2, b2),
        core_ids=list(range(E)),
        trace=trace,
        **trace_kwargs,
    )
    out = np.concatenate([res.results[e]["y"] for e in range(E)], axis=0)
    return out, res


def kernel(inputs, w1, b1, w2, b2):
    out, _ = kernel_run(inputs, w1, b1, w2, b2, trace=False)
    return out
